# revision 23
# baseline (speedup 1.0000x reference)
import sys

for _p in ("/opt/trn_rl_repo", "/opt/trn_rl_repo/concourse"):
    if _p not in sys.path:
        sys.path.insert(0, _p)

import numpy as np
import ml_dtypes
import concourse.bass as bass
import concourse.bacc as bacc
import concourse.mybir as mybir
import concourse.tile as tile

P = 128
D = 512
S = 1600
SP = 1664          # S padded to 13*128
K = 64
NIMG = 4           # images per core
NCORES = 8
NCH = 13           # s-chunks of 128
CA = 8             # chunks in block A
CB = NCH - CA      # chunks in block B
ALPHA = 32.0       # host scale folded into x-hat; cancels in final L2 norm
F32 = mybir.dt.float32
F16 = mybir.dt.float16
F8 = mybir.dt.float8e3   # e3m4
F8E4 = mybir.dt.float8e4  # e4m3
DR = mybir.MatmulPerfMode.DoubleRow
U8 = mybir.dt.uint8
AF = mybir.ActivationFunctionType
OP = mybir.AluOpType
AX = mybir.AxisListType

LN_EIGHTH = -2.0794415416798357  # ln(1/8): global L2 norm is exactly sqrt(K)=8

import os
CFG_DMA = os.environ.get("K_DMA", "rr")        # rr | sp | split
CFG_CONSTQ = os.environ.get("K_CONSTQ", "act")  # act | pool
CFG_WARM = int(os.environ.get("K_WARM", "6"))
CFG_FINE0 = os.environ.get("K_FINE0", "1") == "1"
CFG_DR = os.environ.get("K_DR", "1") == "1"  # DoubleRow fp8e4 GEMMs
XDT = F8E4 if CFG_DR else F8

BLOCKS = ((0, CA), (CA, NCH))
SMBLOCKS = ((0, 4), (4, 8), (8, NCH))


def build():
    nc = bacc.Bacc("TRN2", target_bir_lowering=False, debug=False,
                   enable_asserts=True, num_devices=NCORES)
    # host-relaid layouts (x-hat = ALPHA * x / ||x||_2 per pixel, e3m4):
    #  XN [n, p=d%128, g=d//128, s]  (logits lhsT; s zero-padded to 1664)
    #  XT [n, p=s%128, c=s//128, d]  (agg GEMM rhs; pad rows zero)
    XN_d = nc.dram_tensor("XN", [NIMG, P, 4, SP], U8, kind="ExternalInput").ap()
    XT_d = nc.dram_tensor("XT", [NIMG, P, NCH, D], U8, kind="ExternalInput").ap()
    WT_d = nc.dram_tensor("WT", [P, 4, K], U8 if CFG_DR else F16,
                          kind="ExternalInput").ap()
    CENT_d = nc.dram_tensor("CENT", [K, D], F16, kind="ExternalInput").ap()
    # ONES2 [:, :, 0] = ALPHA everywhere (chunk pairs); [:, 0, 1] = ALPHA on
    # rows 0:64, 0 on rows 64:128 (chunk 12: rows 64:128 are s-padding)
    ONES2_d = nc.dram_tensor("ONES2", [P, 2, 2], U8 if CFG_DR else F16,
                            kind="ExternalInput").ap()
    OUT_d = nc.dram_tensor("OUT", [NIMG, K, D], F16, kind="ExternalOutput").ap()

    with tile.TileContext(nc) as tc:
        with tc.tile_pool(name="const", bufs=1) as cpool, \
             tc.tile_pool(name="xn", bufs=3) as xnpool, \
             tc.tile_pool(name="xt", bufs=3) as xtpool, \
             tc.tile_pool(name="expt", bufs=2) as epool, \
             tc.tile_pool(name="sm", bufs=2) as smpool, \
             tc.tile_pool(name="e2", bufs=28) as e2pool, \
             tc.tile_pool(name="fin", bufs=2) as finpool, \
             tc.tile_pool(name="ps_lg", bufs=2, space=bass.MemorySpace.PSUM) as ps_lg, \
             tc.tile_pool(name="ps_agg", bufs=2, space=bass.MemorySpace.PSUM) as ps_agg, \
             tc.tile_pool(name="ps_as", bufs=2, space=bass.MemorySpace.PSUM) as ps_as:

            # act set 6 = {ln, exp, square, copy, ...}: one table load total
            nc.scalar.add_instruction(mybir.InstLoadActFuncSet(act_func_set_id=6))
            wt = cpool.tile([P, 4, K], U8 if CFG_DR else F16)
            cent = cpool.tile([K, D], F16)
            ones2 = cpool.tile([P, 2, 2], U8 if CFG_DR else F16)
            ln8 = cpool.tile([K, 1], F32)
            nc.vector.memset(ln8[:], LN_EIGHTH)

            # dummy matmuls keep the PE busy through its ~3us p-state ramp
            # while the first image streams in, so real matmuls start at full
            # clock. Results are never read.
            wz = cpool.tile([P, 4 * K], F16)
            nc.vector.memset(wz[:], 0.0)
            warm = ps_lg.tile([8, 4, K], F32, name="warm", tag="lgpA")
            for i in range(max(CFG_WARM, 1)):
                nc.tensor.matmul(warm[:, :, :], wz[:, 0:8], wz[:, :],
                                 start=(i == 0), stop=(i == max(CFG_WARM, 1) - 1))

            xn_t = [None] * NIMG
            xt_t = [None] * NIMG
            expt_t = [None] * NIMG
            sume_t = [None] * NIMG
            rse_t = [None] * NIMG
            lgp_t = {}
            e2_t = {}
            agg_t = [None] * NIMG
            asum_t = [None] * NIMG

            dma_rr = [0]

            def _ldq(is_xt=False):
                # alternate x pieces between the SP and Act DGE queues so
                # neither sequencer's per-DMA issue latency starves the
                # DMA engines
                if CFG_DMA == "sp":
                    return nc.sync
                if CFG_DMA == "split":
                    return nc.scalar if is_xt else nc.sync
                dma_rr[0] ^= 1
                return nc.sync if dma_rr[0] else nc.scalar

            def load_xn(n, fine=False):
                t = xnpool.tile([P, 4, SP], U8, name="xn_t")
                pieces = ((0, 512), (512, 1024), (1024, SP)) if fine \
                    else ((0, CA * P), (CA * P, SP))
                for a, b in pieces:
                    _ldq(False).dma_start(t[:, :, a:b], XN_d[n, :, :, a:b])
                xn_t[n] = t

            def load_xt(n):
                u = xtpool.tile([P, NCH, D], U8, name="xt_t")
                _ldq(True).dma_start(u[:, 0:CA, :], XT_d[n, :, 0:CA, :])
                _ldq(True).dma_start(u[:, CA:NCH, :], XT_d[n, :, CA:NCH, :])
                xt_t[n] = u

            def emit_logits(n, b):
                lo, hi = BLOCKS[b]
                lgp = ps_lg.tile([P, hi - lo, K], F32, name="lgp",
                                 tag=("lgpA" if b == 0 else "lgpB"))
                lgp_t[(n, b)] = lgp
                xn = xn_t[n]
                for j in range(lo, hi):
                    s0 = j * P
                    if CFG_DR:
                        for t in range(2):
                            nc.tensor.matmul(
                                lgp[:, j - lo, :],
                                xn[:, 2 * t:2 * t + 2, s0:s0 + P].bitcast(F8E4),
                                wt[:, 2 * t:2 * t + 2, :].bitcast(F8E4),
                                start=(t == 0), stop=(t == 1), perf_mode=DR)
                    else:
                        for g in range(4):
                            nc.tensor.matmul(lgp[:, j - lo, :],
                                             xn[:, g, s0:s0 + P].bitcast(F8),
                                             wt[:, g, :],
                                             start=(g == 0), stop=(g == 3))

            def emit_softmax(n, b):
                lo, hi = SMBLOCKS[b]
                if b == 0:
                    expt_t[n] = (epool.tile([P, CA, K], F16, name="exptA",
                                            tag="exptA"),
                                 epool.tile([P, CB, K], F16, name="exptB",
                                            tag="exptB"))
                    sume_t[n] = smpool.tile([P, 16], F16, name="sume")
                    rse_t[n] = smpool.tile([P, 16], F32, name="rse")
                sume, rse = sume_t[n], rse_t[n]
                expt = expt_t[n][0 if b < 2 else 1]
                eo = 0 if b < 2 else CA
                lgp = lgp_t[(n, 0 if b < 2 else 1)]
                # logits are ALPHA-scaled by the host relayout; exp rescales
                nc.scalar.activation(out=expt[:, lo - eo:hi - eo, :],
                                     in_=lgp[:, lo - eo:hi - eo, :],
                                     func=AF.Exp,
                                     scale=1.0 / (ALPHA * ALPHA) if CFG_DR
                                     else 1.0 / ALPHA)
                with nc.allow_low_precision("fp16 sumexp of <=64 fp16 terms"):
                    nc.vector.tensor_reduce(out=sume[:, lo:hi],
                                            in_=expt[:, lo - eo:hi - eo, :],
                                            axis=AX.X, op=OP.add)
                nc.vector.reciprocal(rse[:, lo:hi], sume[:, lo:hi])
                for j in range(lo, hi):
                    if CFG_DR:
                        if j % 2 == 0:
                            e2p = e2pool.tile([P, 2, K], F8E4, name="e2t",
                                              tag="e2t")
                            e2_t[(n, j // 2)] = e2p
                        else:
                            e2p = e2_t[(n, j // 2)]
                        dst = e2p[:, j % 2, :]
                    else:
                        dst = e2pool.tile([P, K], F16, name="e2t", tag="e2t")
                        e2_t[(n, j)] = dst
                    nc.vector.tensor_scalar(out=dst,
                                            in0=expt[:, j - eo, :],
                                            scalar1=rse[:, j:j + 1],
                                            scalar2=None, op0=OP.mult)

            def emit_agg(n, b):
                lo, hi = BLOCKS[b]
                if b == 0:
                    agg_t[n] = ps_agg.tile([K, D], F32, name="agg")
                    asum_t[n] = ps_as.tile([K, 8], F32, name="asum")
                agg, asum = agg_t[n], asum_t[n]
                xt = xt_t[n]
                if CFG_DR:
                    for t in range(lo // 2, (hi + 1) // 2):
                        e2p = e2_t[(n, t)]
                        if t == NCH // 2:
                            # solo chunk 12: regular fp8e4 matmul
                            e2_t.pop((n, t))
                            nc.tensor.matmul(agg[:, :], e2p[:, 0, :],
                                             xt[:, NCH - 1, :].bitcast(F8E4),
                                             start=False, stop=True)
                            nc.tensor.matmul(asum[:, 0:1], e2p[:, 0, :],
                                             ones2[:, 0, 1:2].bitcast(F8E4),
                                             start=False, stop=True)
                        else:
                            e2_t.pop((n, t))
                            nc.tensor.matmul(agg[:, :], e2p[:, :, :],
                                             xt[:, 2 * t:2 * t + 2, :]
                                             .bitcast(F8E4),
                                             start=(t == 0), stop=False,
                                             perf_mode=DR)
                            nc.tensor.matmul(asum[:, 0:1], e2p[:, :, :],
                                             ones2[:, :, 0:1].bitcast(F8E4),
                                             start=(t == 0), stop=False,
                                             perf_mode=DR)
                else:
                    for j in range(lo, hi):
                        e2t = e2_t.pop((n, j))
                        nc.tensor.matmul(agg[:, :], e2t[:, :],
                                         xt[:, j, :].bitcast(F8),
                                         start=(j == 0), stop=(j == NCH - 1))
                        oc = 1 if j == NCH - 1 else 0
                        nc.tensor.matmul(asum[:, 0:1], e2t[:, :],
                                         ones2[:, oc, 0 if oc else 0:1]
                                         if False else ones2[:, oc, 0:1],
                                         start=(j == 0), stop=(j == NCH - 1))

            def emit_finale(n, tail=False):
                agg, asum = agg_t[n], asum_t[n]
                # nv = asum*cent - agg = -vlad (ALPHA-scaled; cancels in norm)
                nv = finpool.tile([K, D], F16, name="nv")
                nvsq = finpool.tile([K, D], F16, name="nvsq")
                sc = finpool.tile([K, 4], F32, name="sc")
                ot = finpool.tile([K, D], F16, name="ot")
                nc.vector.scalar_tensor_tensor(out=nv[:, :], in0=cent[:, :],
                                               scalar=asum[:, 0:1],
                                               in1=agg[:, :],
                                               op0=OP.mult, op1=OP.subtract)
                if tail:
                    # tail image: shortest-latency chain, all on DVE/Act
                    nc.vector.scalar_tensor_tensor(out=nvsq[:, :], in0=nv[:, :],
                                                   scalar=1.0, in1=nv[:, :],
                                                   op0=OP.mult, op1=OP.mult,
                                                   accum_out=sc[:, 0:1])
                else:
                    # square+rowsum on Act, final scale on Pool: keeps the DVE
                    # free for the next image's softmax chain
                    nc.scalar.activation(out=nvsq[:, :], in_=nv[:, :],
                                         func=AF.Square, accum_out=sc[:, 0:1])
                nc.scalar.activation(out=sc[:, 1:2], in_=sc[:, 0:1], func=AF.Ln)
                nc.scalar.activation(out=sc[:, 2:3], in_=sc[:, 1:2], func=AF.Exp,
                                     scale=-0.5, bias=ln8[0:K, 0:1])
                if tail:
                    nc.vector.tensor_scalar(out=ot[:, :], in0=nv[:, :],
                                            scalar1=sc[:, 2:3], scalar2=-1.0,
                                            op0=OP.mult, op1=OP.mult)
                else:
                    nc.gpsimd.tensor_scalar(out=ot[:, :], in0=nv[:, :],
                                            scalar1=sc[:, 2:3], scalar2=-1.0,
                                            op0=OP.mult, op1=OP.mult)
                # steady-state outputs ride the Pool SWDGE (no HWDGE slot,
                # never blocks input loads); the tail output takes the lower
                # latency Act HWDGE path since inputs are done by then
                if tail:
                    nc.scalar.dma_start(OUT_d[n, :, :], ot[:, :])
                else:
                    nc.gpsimd.dma_start(OUT_d[n, :, :], ot[:, :])

            # first x piece goes out first on the SP queue; tiny consts ride
            # the Act DGE queue in parallel and land before first use
            _cq = nc.scalar if CFG_CONSTQ == "act" else nc.gpsimd
            _cq.dma_start(wt[:], WT_d[:, :, :])
            load_xn(0, fine=CFG_FINE0)
            _cq.dma_start(ones2[:], ONES2_d[:, :])
            _cq.dma_start(cent[:], CENT_d[:, :])
            load_xt(0)
            load_xn(1)
            load_xt(1)

            for n in range(NIMG):
                # full one-image skew, block-interleaved: image n-1's aggs
                # run from data and e2t produced a full period ago, filling
                # the PE between image n's logit blocks. The last image's
                # logits+softmax are emitted before image n-1's aggs so the
                # whole tail chain runs as soon as its xn lands; only the
                # xt-gated aggs and the finale remain after the spine.
                last = n == NIMG - 1
                if n > 0 and not last:
                    emit_agg(n - 1, 0)
                emit_logits(n, 0)
                emit_softmax(n, 0)
                emit_softmax(n, 1)
                if n > 0 and not last:
                    emit_agg(n - 1, 1)
                emit_logits(n, 1)
                emit_softmax(n, 2)
                if last:
                    emit_agg(n - 1, 0)
                    emit_agg(n - 1, 1)
                if n > 0:
                    emit_finale(n - 1)
                if n == 1:
                    load_xn(2)
                    load_xn(3)
                    load_xt(2)
                if n == 2:
                    load_xt(3)
            emit_agg(NIMG - 1, 0)
            emit_agg(NIMG - 1, 1)
            emit_finale(NIMG - 1, tail=True)
    nc.compile()
    return nc


_NC = None


def _get_nc():
    global _NC
    if _NC is None:
        _NC = build()
    return _NC


def _prep(x, conv_weight, centroids):
    x = np.ascontiguousarray(np.asarray(x), dtype=np.float32)
    w = np.ascontiguousarray(np.asarray(conv_weight), dtype=np.float32)
    c = np.ascontiguousarray(np.asarray(centroids), dtype=np.float32)
    N = x.shape[0]
    xf = x.reshape(N, D, S)
    nrm = np.sqrt((xf * xf).sum(axis=1, keepdims=True))
    xh = (ALPHA * xf / np.maximum(nrm, 1e-12)).astype(np.float32)
    # XN: [N, 4g, 128p, S] -> [N, 128p, 4g, S], zero-pad S to 1664, e3m4
    xp = np.zeros((N, D, SP), dtype=np.float32)
    xp[:, :, :S] = xh
    f8 = ml_dtypes.float8_e4m3fn if CFG_DR else ml_dtypes.float8_e3m4
    xn = np.ascontiguousarray(
        xp.reshape(N, 4, P, SP).transpose(0, 2, 1, 3)
    ).astype(f8).view(np.uint8)
    # XT: [N, D, 13c, 128p] -> [N, 128p, 13c, D] fp8
    xt = np.ascontiguousarray(
        xp.reshape(N, D, NCH, P).transpose(0, 3, 2, 1)
    ).astype(f8).view(np.uint8)
    wTf = np.ascontiguousarray(w.reshape(K, 4, P).transpose(2, 1, 0))
    if CFG_DR:
        wT = (ALPHA * wTf).astype(ml_dtypes.float8_e4m3fn).view(np.uint8)
    else:
        wT = wTf.astype(np.float16)
    c16 = c.astype(np.float16)
    if CFG_DR:
        ones2 = np.zeros((P, 2, 2), dtype=np.float32)
        ones2[:, :, 0] = ALPHA
        ones2[0:K, 0, 1] = ALPHA
        ones2 = ones2.astype(ml_dtypes.float8_e4m3fn).view(np.uint8)
    else:
        ones2 = np.zeros((P, 2, 2), dtype=np.float16)
        ones2[:, :, 0] = ALPHA
        ones2[0:K, 0, 1] = ALPHA
    in_maps = [{"XN": np.ascontiguousarray(xn[NIMG * i:NIMG * (i + 1)]),
                "XT": np.ascontiguousarray(xt[NIMG * i:NIMG * (i + 1)]),
                "WT": wT, "CENT": c16, "ONES2": ones2} for i in range(NCORES)]
    return in_maps


def _run(x, conv_weight, centroids, trace=False):
    from concourse import bass_utils
    nc = _get_nc()
    in_maps = _prep(x, conv_weight, centroids)
    res = bass_utils.run_bass_kernel_spmd(nc, in_maps,
                                          core_ids=list(range(NCORES)),
                                          trace=trace)
    out = np.concatenate(
        [np.asarray(res.results[i]["OUT"]).astype(np.float32).reshape(NIMG, K * D)
         for i in range(NCORES)], axis=0)
    return out, getattr(res, "exec_time_ns", None)


def kernel(x, conv_weight, centroids):
    out, _ = _run(x, conv_weight, centroids, trace=False)
    return out
